# revision 10
# baseline (speedup 1.0000x reference)
"""Causal single-head attention on 8 trn2 NeuronCores (Bass/Tile).

Problem: X [4, 2048, 2048] f32, W_Q/W_K/W_V [2048, 256] f32.
  Z = softmax(mask((X@W_Q)(X@W_K)^T / sqrt(256))) @ (X@W_V)

Sharding: 8 cores = 4 batches x 2 query-stripes. Core (b, s) handles the
queries of batch b at token positions == s (mod 2) -- striping balances the
causal attention work exactly across the two cores of a batch, and makes the
per-core mask structure uniform (the only cross-core difference is whether
the remote stripe's same-index key is visible, which we feed in as a tiny
per-core additive diagonal mask).

On-chip layout: the host pre-transposes X to feature-major XT [d_model, rows]
(bf16, partition-major so every DMA is contiguous per partition), so every
projection is a natural PE matmul (contraction on the partition axis).
Attention scores are computed transposed, S^T = K Q^T in [keys, queries]
tiles; exp runs on ScalarE straight out of PSUM (no max subtraction -- score
scale here is ~N(0, 1.8), exp stays well inside f32 range); the softmax
denominator falls out of the P^T @ V_aug matmul via a ones-column appended to
V. All matmuls are bf16 (f32 matmul is 4x slower on the PE); accumulation is
f32 in PSUM; final normalize is f32.

Each core projects K/V only for its own stripe; the other stripe's K/V
arrives via a pairwise AllGather (two collectives, K^T first, so the exchange
overlaps the Q projection and the local-segment attention). The attention is
phased so all collective-independent PE work is issued first.

kernel() takes the FULL inputs and returns the FULL output.
"""

from contextlib import ExitStack

import numpy as np
import ml_dtypes

import concourse.bass as bass
import concourse.tile as tile
from concourse import bacc, mybir
from concourse.bass_utils import run_bass_kernel_spmd

BF16 = mybir.dt.bfloat16
F32 = mybir.dt.float32

B, L, D, DK, DV = 4, 2048, 2048, 256, 256
LQ = L // 2          # queries per core (one stripe)
NT = D // 128        # 16 d_model tiles
KSEG = LQ // 128     # 8 key tiles per segment
CHUNK = 512          # query free-dim chunk for the scores matmul
NCHUNK = LQ // CHUNK  # 2
SCALE = 1.0 / float(np.sqrt(DK))
MASK = -1e9

USE_COLLECTIVE = True


def build_kernel(use_collective: bool):
    nc = bacc.Bacc("TRN2", target_bir_lowering=False, debug=False, num_devices=8)

    xcols = LQ if use_collective else L

    xt_ext = nc.declare_dram_parameter("XT", [128, NT, xcols], BF16, isOutput=False)
    wq_ext = nc.declare_dram_parameter("WQ", [128, NT, DK], BF16, isOutput=False)
    wk_ext = nc.declare_dram_parameter("WK", [128, NT, DK], BF16, isOutput=False)
    wv_ext = nc.declare_dram_parameter("WV", [128, NT, DV], BF16, isOutput=False)
    diagr_ext = nc.declare_dram_parameter("DIAGR", [128, 128], F32, isOutput=False)
    selw_ext = nc.declare_dram_parameter("SELW", [128, 2], F32, isOutput=False)
    out_ext = nc.declare_dram_parameter("OUT", [LQ, DV], F32, isOutput=True)

    # DRAM bounce buffers for the pairwise K/V exchange (bf16).
    vcols = KSEG * (DV + 1)  # 2056
    if use_collective:
        kt_bounce = nc.dram_tensor("kt_bounce", [128, 2 * LQ], BF16)
        kt_gat = nc.dram_tensor("kt_gat", [2, 128, 2 * LQ], BF16)
        v_bounce = nc.dram_tensor("v_bounce", [128, vcols], BF16)
        v_gat = nc.dram_tensor("v_gat", [2, 128, vcols], BF16)

    with tile.TileContext(nc) as tc, ExitStack() as ctx:
        const = ctx.enter_context(tc.tile_pool(name="const", bufs=1))
        xt_pool = ctx.enter_context(tc.tile_pool(name="xt", bufs=1))
        psum = ctx.enter_context(tc.tile_pool(name="psum", bufs=3, space="PSUM"))
        opsum = ctx.enter_context(tc.tile_pool(name="opsum", bufs=4, space="PSUM"))
        ptile_pool = ctx.enter_context(tc.tile_pool(name="ptile", bufs=3))
        small = ctx.enter_context(tc.tile_pool(name="small", bufs=4))

        # ---- input DMAs: weights on the gpsimd (SWDGE) queue, XT on the
        # sync (HWDGE) queue so they stream in parallel; first XT group is
        # small so the first K^T matmul can start early.
        wk = const.tile([128, NT, DK], BF16)
        nc.gpsimd.dma_start(wk[:], wk_ext[:, :, :])
        xt_sb = xt_pool.tile([128, NT, xcols], BF16, name="xt_sb")
        xt_groups = [(0, 2), (2, 4), (6, 4), (10, 3), (13, 3)]
        for g0, gn in xt_groups:
            nc.sync.dma_start(
                xt_sb[:, g0:g0 + gn, :], xt_ext[:, g0:g0 + gn, :]
            )
        xt = [xt_sb[:, dt, :] for dt in range(NT)]
        wv = const.tile([128, NT, DV], BF16)
        nc.gpsimd.dma_start(wv[:], wv_ext[:, :, :])
        wq = const.tile([128, NT, DK], BF16)
        nc.gpsimd.dma_start(wq[:], wq_ext[:, :, :])
        diagr = const.tile([128, 128], F32)
        nc.gpsimd.dma_start(diagr[:], diagr_ext.ap())
        selw = const.tile([128, 2], F32)
        nc.gpsimd.dma_start(selw[:], selw_ext.ap())

        # masks[seg][j], j = kb - 4c for the diagonal band of each 512-query
        # chunk: additive f32 [128 keys, 512 queries] tiles.
        #   visible iff (128*i + y2) >= (128*j + x)   (i = query subtile 0..3)
        masks = []
        for seg in range(2):
            row = []
            for j in range(4):
                m = const.tile([128, CHUNK], F32, name=f"mask_{seg}_{j}")
                nc.gpsimd.memset(m[:], 0.0)
                nc.gpsimd.affine_select(
                    out=m[:],
                    in_=m[:],
                    compare_op=mybir.AluOpType.is_ge,
                    fill=MASK,
                    base=-128 * j,
                    pattern=[[128, 4], [1, 128]],
                    channel_multiplier=-1,
                )
                if seg == 1:
                    # remote stripe: diagonal visibility differs by core parity
                    nc.vector.tensor_add(
                        m[:, j * 128:(j + 1) * 128],
                        m[:, j * 128:(j + 1) * 128],
                        diagr[:],
                    )
                row.append(m)
            masks.append(row)

        rg = [[0, 1], [2, 3], [4, 5], [6, 7]]

        # ---- K^T projection (local stripe): [128, 2(m), LQ] bf16 ----------
        kt_loc = const.tile([128, 2, LQ], BF16)
        kt_rem = const.tile([128, 2, LQ], BF16)
        kt_all = [kt_loc, kt_rem]

        def project_kt(dst, seg):
            for m in range(2):
                for n in range(LQ // CHUNK):
                    col0 = seg * LQ + n * CHUNK
                    ps = psum.tile([128, CHUNK], F32, name="ps", tag="s")
                    for dt in range(NT):
                        nc.tensor.matmul(
                            ps[:],
                            wk[:, dt, m * 128:(m + 1) * 128],
                            xt[dt][:, col0:col0 + CHUNK],
                            start=(dt == 0),
                            stop=(dt == NT - 1),
                        )
                    nc.scalar.copy(dst[:, m, n * CHUNK:(n + 1) * CHUNK], ps[:])

        project_kt(kt_loc, 0)
        if use_collective:
            nc.sync.dma_start(kt_bounce[:, :], kt_loc.rearrange("p m q -> p (m q)"))
            nc.gpsimd.collective_compute(
                "AllGather", mybir.AluOpType.bypass, replica_groups=rg,
                ins=[kt_bounce.ap()], outs=[kt_gat.ap()],
            )

        # ---- V projection (local stripe): [128, KSEG, 257] bf16 ------------
        v_loc = const.tile([128, KSEG, DV + 1], BF16)
        v_rem = const.tile([128, KSEG, DV + 1], BF16)
        v_all = [v_loc, v_rem]

        def project_v(dst, seg):
            nc.vector.memset(dst[:, :, DV:DV + 1], 1.0)
            for rt in range(KSEG):
                col0 = seg * LQ + rt * 128
                ps = psum.tile([128, DV], F32, name="ps", tag="s")
                for dt in range(NT):
                    nc.tensor.matmul(
                        ps[:],
                        xt[dt][:, col0:col0 + 128],
                        wv[:, dt, :],
                        start=(dt == 0),
                        stop=(dt == NT - 1),
                    )
                nc.scalar.copy(dst[:, rt, 0:DV], ps[:])

        project_v(v_loc, 0)
        if use_collective:
            nc.sync.dma_start(v_bounce[:, :], v_loc.rearrange("p t c -> p (t c)"))
            nc.gpsimd.collective_compute(
                "AllGather", mybir.AluOpType.bypass, replica_groups=rg,
                ins=[v_bounce.ap()], outs=[v_gat.ap()],
            )

        # ---- Q^T projection: [128, 2(m), LQ] bf16 --------------------------
        qt = const.tile([128, 2, LQ], BF16)
        for m in range(2):
            for n in range(LQ // CHUNK):
                ps = psum.tile([128, CHUNK], F32, name="ps", tag="s")
                for dt in range(NT):
                    nc.tensor.matmul(
                        ps[:],
                        wq[:, dt, m * 128:(m + 1) * 128],
                        xt[dt][:, n * CHUNK:n * CHUNK + CHUNK],
                        start=(dt == 0),
                        stop=(dt == NT - 1),
                    )
                nc.scalar.copy(qt[:, m, n * CHUNK:(n + 1) * CHUNK], ps[:])

        # ---- fetch gathered K/V, select the pair peer's half ---------------
        if use_collective:
            ktg0 = const.tile([128, 2 * LQ], BF16)
            nc.sync.dma_start(ktg0[:], kt_gat[0, :, :])
            ktg1 = const.tile([128, 2 * LQ], BF16)
            nc.sync.dma_start(ktg1[:], kt_gat[1, :, :])
            ktt = const.tile([128, 2 * LQ], BF16)
            kt_rem_flat = kt_rem.rearrange("p m q -> p (m q)")
            nc.vector.tensor_scalar_mul(kt_rem_flat[:], ktg0[:], selw[:, 0:1])
            nc.vector.tensor_scalar_mul(ktt[:], ktg1[:], selw[:, 1:2])
            nc.vector.tensor_add(kt_rem_flat[:], kt_rem_flat[:], ktt[:])

            vg0 = const.tile([128, vcols], BF16)
            nc.sync.dma_start(vg0[:], v_gat[0, :, :])
            vg1 = const.tile([128, vcols], BF16)
            nc.sync.dma_start(vg1[:], v_gat[1, :, :])
            vt = const.tile([128, vcols], BF16)
            v_rem_flat = v_rem.rearrange("p t c -> p (t c)")
            nc.vector.tensor_scalar_mul(v_rem_flat[:], vg0[:], selw[:, 0:1])
            nc.vector.tensor_scalar_mul(vt[:], vg1[:], selw[:, 1:2])
            nc.vector.tensor_add(v_rem_flat[:], v_rem_flat[:], vt[:])
        else:
            project_kt(kt_rem, 1)
            project_v(v_rem, 1)

        # ---- attention -----------------------------------------------------
        # S^T tiles [128 keys, 512 queries]; P^T = exp(S^T/16 + mask);
        # O_aug[q] [128 q, 257] accumulates P^T.T @ V_aug over (seg, kb).
        # Phased so every collective-independent PE op issues first:
        #   ph1: chunk0 seg0 scores+AV   ph2: chunk1 seg0 scores -> p_store
        #   ph3: chunk0 seg1 + normalize ph4: chunk1 AV(seg0) + seg1 + norm
        def scores_exp(c, seg, kb, p_out):
            s_ps = psum.tile([128, CHUNK], F32, name="ps", tag="s")
            for m in range(2):
                nc.tensor.matmul(
                    s_ps[:],
                    kt_all[seg][:, m, kb * 128:(kb + 1) * 128],
                    qt[:, m, c * CHUNK:(c + 1) * CHUNK],
                    start=(m == 0),
                    stop=(m == 1),
                )
            j = kb - 4 * c
            if j >= 0:
                nc.vector.tensor_add(s_ps[:], s_ps[:], masks[seg][j][:])
            nc.scalar.activation(
                p_out[:], s_ps[:], mybir.ActivationFunctionType.Exp, scale=SCALE
            )

        def av(c, seg, kb, p, o_ps):
            for q in range(4):
                ti = 4 * c + q
                if kb > ti:
                    continue
                nc.tensor.matmul(
                    o_ps[q][:],
                    p[:, q * 128:(q + 1) * 128],
                    v_all[seg][:, kb, :],
                    start=(seg == 0 and kb == 0),
                    stop=(seg == 1 and kb == ti),
                )

        def normalize_q(c, q, o_ps):
            recip = small.tile([128, 1], F32, name="recip")
            nc.vector.reciprocal(recip[:], o_ps[q][:, DV:DV + 1])
            o_sb = small.tile([128, DV], F32, name="o_sb")
            nc.vector.tensor_scalar_mul(o_sb[:], o_ps[q][:, 0:DV], recip[:])
            r0 = (4 * c + q) * 128
            nc.sync.dma_start(out_ext[r0:r0 + 128, :], o_sb[:])

        # ph1: chunk 0, local segment
        o_ps0 = [opsum.tile([128, DV + 1], F32, name="o_ps", tag="o") for _ in range(4)]
        for kb in range(4):
            p = ptile_pool.tile([128, CHUNK], BF16, name="p")
            scores_exp(0, 0, kb, p)
            av(0, 0, kb, p, o_ps0)

        # ph2: chunk 1, local segment scores (AV deferred until chunk 0 frees
        # its PSUM banks); keeps the PE busy while the K/V exchange flies.
        p_store = [
            const.tile([128, CHUNK], BF16, name=f"p_store_{kb}") for kb in range(8)
        ]
        for kb in range(8):
            scores_exp(1, 0, kb, p_store[kb])

        # ph3: chunk 0, remote segment; normalize each q right after its
        # accumulation stops (kb == ti)
        for kb in range(4):
            p = ptile_pool.tile([128, CHUNK], BF16, name="p")
            scores_exp(0, 1, kb, p)
            av(0, 1, kb, p, o_ps0)
            normalize_q(0, kb, o_ps0)

        # ph4: chunk 1: AV over stored local P, then remote segment
        o_ps1 = [opsum.tile([128, DV + 1], F32, name="o_ps", tag="o") for _ in range(4)]
        for kb in range(8):
            av(1, 0, kb, p_store[kb], o_ps1)
        for kb in range(8):
            p = ptile_pool.tile([128, CHUNK], BF16, name="p")
            scores_exp(1, 1, kb, p)
            av(1, 1, kb, p, o_ps1)
            if kb >= 4:
                normalize_q(1, kb - 4, o_ps1)

    nc.finalize()
    return nc


_CACHED = {}


def _get_kernel(use_collective: bool):
    if use_collective not in _CACHED:
        _CACHED[use_collective] = build_kernel(use_collective)
    return _CACHED[use_collective]


def _prepare_in_maps(X, W_Q, W_K, W_V, use_collective):
    def wlayout(W):
        # w[p, dt, c] = W[dt*128 + p, c]
        n = W.shape[1]
        return np.ascontiguousarray(
            W.reshape(NT, 128, n).transpose(1, 0, 2)
        ).astype(ml_dtypes.bfloat16)

    wq = wlayout(W_Q)
    wk = wlayout(W_K)
    wv = wlayout(W_V)

    in_maps = []
    for core in range(8):
        b, s = core // 2, core % 2
        # partition-major layout: xt[p, dt, r] = X[b, stripe r, dt*128 + p]
        loc = X[b, s::2, :].reshape(LQ, NT, 128).transpose(2, 1, 0)
        if use_collective:
            xt = np.ascontiguousarray(loc).astype(ml_dtypes.bfloat16)
        else:
            remo = X[b, 1 - s::2, :].reshape(LQ, NT, 128).transpose(2, 1, 0)
            xt = np.concatenate([loc, remo], axis=2).astype(ml_dtypes.bfloat16)
        diagr = np.zeros((128, 128), np.float32)
        if s == 0:
            np.fill_diagonal(diagr, MASK)
        selw = np.zeros((128, 2), np.float32)
        selw[:, 1 - s] = 1.0  # pick the pair peer's slot from the gather
        in_maps.append(
            {"XT": xt, "WQ": wq, "WK": wk, "WV": wv, "DIAGR": diagr, "SELW": selw}
        )
    return in_maps


def _assemble(results):
    Z = np.empty((B, L, DV), np.float32)
    for core in range(8):
        b, s = core // 2, core % 2
        Z[b, s::2, :] = results[core]["OUT"]
    return Z


def kernel(X, W_Q, W_K, W_V):
    nc = _get_kernel(USE_COLLECTIVE)
    in_maps = _prepare_in_maps(X, W_Q, W_K, W_V, USE_COLLECTIVE)
    res = run_bass_kernel_spmd(nc, in_maps, core_ids=list(range(8)))
    return _assemble(res.results)
